# revision 14
# baseline (speedup 1.0000x reference)
"""MCR2 variational loss on 8 Trainium2 NeuronCores.

Math (reference):
  loss_R   = 0.5 * logdet(I + d/(n*eps) * Z.T @ Z)
  loss_Rc  = 0.5 * sum_k(trPi_k * sum_p log1p(d/(trPi_k*eps) * relu(A)_kp)) / n
  loss_reg = 0.5 * sum_k || G_k - Un diag(relu(A)_k) Un.T ||_F^2,
             G_k = Z.T diag(Pi[:,k]) Z
  out = (-(loss_R - loss_Rc - mu*loss_reg), loss_R, loss_Rc, loss_reg)

The only O(n) work is the 11 Grams (10 masked + 1 full). Device strategy:
shard rows across 8 cores; per core, for each 128-row chunk of Z, build all
10 weighted copies W_k = Pi[:,k] * Z with a single DVE tensor_tensor whose
second operand is a broadcast AP over a bf16 [w,w] pair tile (pair packing
keeps the DVE 2x perf mode), then accumulate Z.T @ [W_0..W_9 | Z] into PSUM
on the tensor engine (bf16 x bf16 -> fp32). The fp32->bf16 cast of Z runs
on the scalar engine per 7-chunk group. Per-core partial Grams [128 x 1408]
are summed on host; the O(k*d^2) epilogue (slogdet etc.) runs on host in
float64.
"""

import sys

if "/opt/trn_rl_repo" not in sys.path:
    sys.path.insert(0, "/opt/trn_rl_repo")

import ml_dtypes
import numpy as np

import concourse.bacc as bacc
import concourse.mybir as mybir
import concourse.tile as tile
from concourse import bass_utils

# Problem constants (hardcoded per harness contract).
N, D, K = 100000, 128, 10
EPS, MU = 0.5, 1.0
N_CORES = 8
CHUNKS = 98                    # 128-row chunks per core
SHARD = CHUNKS * 128           # 12544 rows per core
NPAD = SHARD * N_CORES         # 100352 (zero-padded; zero rows contribute 0)
GROUP = 7                      # chunks per staged DMA group
N_GROUPS = CHUNKS // GROUP     # 14
NCLS = K + 1                   # 10 masked Grams + 1 full Gram

_NC_CACHE = None


def _build_nc():
    f32 = mybir.dt.float32
    bf16 = mybir.dt.bfloat16
    Copy = mybir.ActivationFunctionType.Copy

    nc = bacc.Bacc("TRN2", target_bir_lowering=False, debug=False)
    Zs = nc.dram_tensor("Zs", [SHARD, D], f32, kind="ExternalInput")
    # Pi, host-preprocessed: [p, chunk, class, 2] bf16 with the weight
    # duplicated in the last axis so the DVE reads an aligned [w,w] pair.
    KD = K - 1  # classes 0..8 weighted on DVE; class 9 on ACT
    Pb = nc.dram_tensor("Pb", [128, CHUNKS, KD, 2], bf16, kind="ExternalInput")
    Pf = nc.dram_tensor("Pf", [128, CHUNKS], f32, kind="ExternalInput")
    G = nc.dram_tensor("G", [D, NCLS * D], f32, kind="ExternalOutput")

    with tile.TileContext(nc) as tc:
        with (
            tc.tile_pool(name="zst", bufs=3) as zpool,
            tc.tile_pool(name="zbf", bufs=3) as zbpool,
            tc.tile_pool(name="wgt", bufs=2) as wpool,
            tc.tile_pool(name="pi", bufs=1) as pipool,
            tc.tile_pool(name="res", bufs=1) as opool,
            tc.tile_pool(name="ps", bufs=1, space="PSUM") as pspool,
        ):
            pib = pipool.tile([128, CHUNKS, KD, 2], bf16, name="pib")
            nc.sync.dma_start(pib[:], Pb[:])
            pif = pipool.tile([128, CHUNKS], f32, name="pif")
            nc.sync.dma_start(pif[:], Pf[:])

            psA = pspool.tile([128, 512], f32, name="psA")
            psB = pspool.tile([128, 512], f32, name="psB")
            psC = pspool.tile([128, 256], f32, name="psC")
            psD = pspool.tile([128, 128], f32, name="psD")

            Zr = Zs.rearrange("(c p) d -> p c d", p=128)

            # Small first group so compute starts early; small last group
            # to shrink the pipeline drain.
            sizes = [1] + [GROUP] * 13 + [6]
            assert sum(sizes) == CHUNKS

            start_c = 0
            for sz in sizes:
                s0 = start_c
                start_c += sz
                zst = zpool.tile([128, sz, D], f32, name="zst", tag="zst")
                nc.sync.dma_start(zst[:], Zr[:, s0:s0 + sz, :])
                zb = zbpool.tile([128, sz, D], bf16, name="zb", tag="zb")
                nc.scalar.activation(zb[:], zst[:], Copy)

                # Fused weighted-copy for classes 0..8 over the whole group:
                #   wg[p, c, k, 2r+t] = zb[p, c, 2r+t] * pib[p, s0+c, k]
                # bf16 [w,w] pair packing keeps the DVE 2x perf mode.
                wg = wpool.tile([128, sz, K * D], bf16, name="wg", tag="wg")
                z_bc = zb[:].unsqueeze(2).broadcast_to([128, sz, KD, D])
                pi_bc = (
                    pib[:, s0:s0 + sz, :, :]
                    .unsqueeze(3)
                    .broadcast_to([128, sz, KD, 64, 2])
                )
                w5 = wg[:, :, 0:KD * D].rearrange(
                    "p c (k r t) -> p c k r t", k=KD, t=2
                )
                z5 = z_bc.rearrange("p c k (r t) -> p c k r t", t=2)
                nc.vector.tensor_mul(w5, z5, pi_bc)

                for c in range(sz):
                    idx = s0 + c
                    first = idx == 0
                    last = idx == CHUNKS - 1
                    zc = zb[:, c, :]
                    w = wg[:, c, :]
                    # class 9 weighted copy on the scalar engine
                    nc.scalar.activation(
                        w[:, KD * D:K * D], zc, Copy,
                        bias=0.0, scale=pif[:, idx:idx + 1],
                    )
                    nc.tensor.matmul(psA[:], zc, w[:, 0:512], start=first, stop=last)
                    nc.tensor.matmul(psB[:], zc, w[:, 512:1024], start=first, stop=last)
                    nc.tensor.matmul(psC[:], zc, w[:, 1024:1280], start=first, stop=last)
                    nc.tensor.matmul(psD[:], zc, zc, start=first, stop=last)

            out = opool.tile([128, NCLS * D], f32, name="out")
            nc.vector.tensor_copy(out[:, 0:512], psA[:])
            nc.scalar.copy(out[:, 512:1024], psB[:])
            nc.vector.tensor_copy(out[:, 1024:1280], psC[:])
            nc.scalar.copy(out[:, 1280:1408], psD[:])
            nc.sync.dma_start(G[:], out[:])

    nc.compile()
    return nc


def _get_nc():
    global _NC_CACHE
    if _NC_CACHE is None:
        _NC_CACHE = _build_nc()
    return _NC_CACHE


def _make_in_maps(Z, Pi):
    Zpad = np.zeros((NPAD, D), np.float32)
    Zpad[:N] = Z
    Pipad = np.zeros((NPAD, K), np.float32)
    Pipad[:N] = Pi
    in_maps = []
    for i in range(N_CORES):
        zs = np.ascontiguousarray(Zpad[i * SHARD:(i + 1) * SHARD])
        pt = (
            Pipad[i * SHARD:(i + 1) * SHARD, 0:K - 1]
            .reshape(CHUNKS, 128, K - 1)
            .transpose(1, 0, 2)
            .astype(ml_dtypes.bfloat16)
        )
        pb = np.ascontiguousarray(np.repeat(pt[..., None], 2, axis=-1))
        pf = np.ascontiguousarray(
            Pipad[i * SHARD:(i + 1) * SHARD, K - 1]
            .reshape(CHUNKS, 128)
            .T
        )
        in_maps.append({"Zs": zs, "Pb": pb, "Pf": pf})
    return in_maps


def _run_device(in_maps, trace=False, tmpdir=None):
    nc = _get_nc()
    return bass_utils.run_bass_kernel_spmd(
        nc, in_maps, core_ids=list(range(N_CORES)), trace=trace, tmpdir=tmpdir
    )


def _epilogue(G_all, Pi, A, U):
    """Host epilogue in float64. G_all: [128, 1408] summed partial Grams."""
    G_all = G_all.astype(np.float64)
    Gk = np.stack(
        [G_all[:, k * D:(k + 1) * D] for k in range(K)]
    )                                   # [K, D, D] masked Grams
    Gram = G_all[:, K * D:(K + 1) * D]  # [D, D] full Gram

    d_f = float(D)
    n_f = float(N)

    Mat = np.eye(D, dtype=np.float64) + (d_f / (n_f * EPS)) * Gram
    _, logdet = np.linalg.slogdet(Mat)
    loss_R = 0.5 * logdet

    trPi = Pi.astype(np.float64).sum(axis=0)            # [K]
    scalar = d_f / (trPi * EPS)
    Ar = np.maximum(A.astype(np.float64), 0.0)          # [K, D]
    logdets = np.log1p(scalar[:, None] * Ar).sum(axis=1)
    loss_Rc = 0.5 * np.sum(logdets * trPi) / n_f

    norms = np.maximum(np.linalg.norm(U, axis=0, keepdims=True), 1e-12)
    Un = (U / norms).astype(np.float64)
    M = np.einsum("dp,kp,ep->kde", Un, Ar, Un)
    loss_reg = 0.5 * np.sum((Gk - M) ** 2)

    loss_obj = loss_R - loss_Rc - MU * loss_reg
    return (
        np.float32(-loss_obj),
        np.float32(loss_R),
        np.float32(loss_Rc),
        np.float32(loss_reg),
    )


def kernel(Z, Pi, A, U):
    Z = np.asarray(Z, dtype=np.float32)
    Pi = np.asarray(Pi, dtype=np.float32)
    A = np.asarray(A, dtype=np.float32)
    U = np.asarray(U, dtype=np.float32)

    in_maps = _make_in_maps(Z, Pi)
    res = _run_device(in_maps)
    G_all = np.zeros((D, NCLS * D), np.float64)
    for i in range(N_CORES):
        G_all += res.results[i]["G"]
    return _epilogue(G_all, Pi, A, U)


# revision 18
# speedup vs baseline: 1.0821x; 1.0821x over previous
"""MCR2 variational loss on 8 Trainium2 NeuronCores.

Math (reference):
  loss_R   = 0.5 * logdet(I + d/(n*eps) * Z.T @ Z)
  loss_Rc  = 0.5 * sum_k(trPi_k * sum_p log1p(d/(trPi_k*eps) * relu(A)_kp)) / n
  loss_reg = 0.5 * sum_k || G_k - Un diag(relu(A)_k) Un.T ||_F^2,
             G_k = Z.T diag(Pi[:,k]) Z
  out = (-(loss_R - loss_Rc - mu*loss_reg), loss_R, loss_Rc, loss_reg)

The only O(n) work is the 11 Grams (10 masked + 1 full). Device strategy:
shard rows across 8 cores; per core, for each 128-row chunk of Z, build all
10 weighted copies W_k = Pi[:,k] * Z with a single DVE tensor_tensor whose
second operand is a broadcast AP over a bf16 [w,w] pair tile (pair packing
keeps the DVE 2x perf mode), then accumulate Z.T @ [W_0..W_9 | Z] into PSUM
on the tensor engine (bf16 x bf16 -> fp32). The fp32->bf16 cast of Z runs
on the scalar engine per 7-chunk group. Per-core partial Grams [128 x 1408]
are summed on host; the O(k*d^2) epilogue (slogdet etc.) runs on host in
float64.
"""

import sys

if "/opt/trn_rl_repo" not in sys.path:
    sys.path.insert(0, "/opt/trn_rl_repo")

import ml_dtypes
import numpy as np

import concourse.bacc as bacc
import concourse.mybir as mybir
import concourse.tile as tile
from concourse import bass_utils

# Problem constants (hardcoded per harness contract).
N, D, K = 100000, 128, 10
EPS, MU = 0.5, 1.0
N_CORES = 8
CHUNKS = 98                    # 128-row chunks per core
SHARD = CHUNKS * 128           # 12544 rows per core
NPAD = SHARD * N_CORES         # 100352 (zero-padded; zero rows contribute 0)
GROUP = 7                      # chunks per staged DMA group
N_GROUPS = CHUNKS // GROUP     # 14
NCLS = K + 1                   # 10 masked Grams + 1 full Gram

_NC_CACHE = None


def _build_nc():
    f32 = mybir.dt.float32
    bf16 = mybir.dt.bfloat16
    Copy = mybir.ActivationFunctionType.Copy

    nc = bacc.Bacc("TRN2", target_bir_lowering=False, debug=False)
    Zs = nc.dram_tensor("Zs", [SHARD, D], bf16, kind="ExternalInput")
    # Pi, host-preprocessed: [p, chunk, class, 2] bf16 with the weight
    # duplicated in the last axis so the DVE reads an aligned [w,w] pair.
    KD = K - 1  # classes 0..8 weighted on DVE; class 9 on ACT
    Pb = nc.dram_tensor("Pb", [128, CHUNKS, KD, 2], bf16, kind="ExternalInput")
    Pf = nc.dram_tensor("Pf", [128, CHUNKS], f32, kind="ExternalInput")
    G = nc.dram_tensor("G", [D, NCLS * D], f32, kind="ExternalOutput")

    with tile.TileContext(nc) as tc:
        with (
            tc.tile_pool(name="zbf", bufs=3) as zbpool,
            tc.tile_pool(name="wgt", bufs=2) as wpool,
            tc.tile_pool(name="pi", bufs=1) as pipool,
            tc.tile_pool(name="res", bufs=1) as opool,
            tc.tile_pool(name="ps", bufs=1, space="PSUM") as pspool,
        ):
            pib = pipool.tile([128, CHUNKS, KD, 2], bf16, name="pib")
            nc.sync.dma_start(pib[:], Pb[:])
            pif = pipool.tile([128, CHUNKS], f32, name="pif")
            nc.sync.dma_start(pif[:], Pf[:])

            psA = pspool.tile([128, 512], f32, name="psA")
            psB = pspool.tile([128, 512], f32, name="psB")
            psC = pspool.tile([128, 256], f32, name="psC")
            psD = pspool.tile([128, 128], f32, name="psD")

            Zr = Zs.rearrange("(c p) d -> p c d", p=128)

            # Small first group so compute starts early; small last group
            # to shrink the pipeline drain.
            sizes = [1] + [GROUP] * 13 + [6]
            assert sum(sizes) == CHUNKS

            start_c = 0
            for sz in sizes:
                s0 = start_c
                start_c += sz
                zb = zbpool.tile([128, sz, D], bf16, name="zb", tag="zb")
                nc.sync.dma_start(zb[:], Zr[:, s0:s0 + sz, :])

                # Fused weighted-copy for classes 0..8 over the whole group:
                #   wg[p, c, k, 2r+t] = zb[p, c, 2r+t] * pib[p, s0+c, k]
                # bf16 [w,w] pair packing keeps the DVE 2x perf mode.
                wg = wpool.tile([128, sz, K * D], bf16, name="wg", tag="wg")
                z_bc = zb[:].unsqueeze(2).broadcast_to([128, sz, KD, D])
                pi_bc = (
                    pib[:, s0:s0 + sz, :, :]
                    .unsqueeze(3)
                    .broadcast_to([128, sz, KD, 64, 2])
                )
                w5 = wg[:, :, 0:KD * D].rearrange(
                    "p c (k r t) -> p c k r t", k=KD, t=2
                )
                z5 = z_bc.rearrange("p c k (r t) -> p c k r t", t=2)
                nc.vector.tensor_mul(w5, z5, pi_bc)

                for c in range(sz):
                    idx = s0 + c
                    first = idx == 0
                    last = idx == CHUNKS - 1
                    zc = zb[:, c, :]
                    w = wg[:, c, :]
                    # class 9 weighted copy on the scalar engine
                    nc.scalar.activation(
                        w[:, KD * D:K * D], zc, Copy,
                        bias=0.0, scale=pif[:, idx:idx + 1],
                    )
                    nc.tensor.matmul(psA[:], zc, w[:, 0:512], start=first, stop=last)
                    nc.tensor.matmul(psB[:], zc, w[:, 512:1024], start=first, stop=last)
                    nc.tensor.matmul(psC[:], zc, w[:, 1024:1280], start=first, stop=last)
                    nc.tensor.matmul(psD[:], zc, zc, start=first, stop=last)

            out = opool.tile([128, NCLS * D], f32, name="out")
            nc.vector.tensor_copy(out[:, 0:512], psA[:])
            nc.scalar.copy(out[:, 512:1024], psB[:])
            nc.vector.tensor_copy(out[:, 1024:1280], psC[:])
            nc.scalar.copy(out[:, 1280:1408], psD[:])
            nc.sync.dma_start(G[:], out[:])

    nc.compile()
    return nc


def _get_nc():
    global _NC_CACHE
    if _NC_CACHE is None:
        _NC_CACHE = _build_nc()
    return _NC_CACHE


def _make_in_maps(Z, Pi):
    Zpad = np.zeros((NPAD, D), ml_dtypes.bfloat16)
    Zpad[:N] = Z.astype(ml_dtypes.bfloat16)
    Pipad = np.zeros((NPAD, K), np.float32)
    Pipad[:N] = Pi
    in_maps = []
    for i in range(N_CORES):
        zs = np.ascontiguousarray(Zpad[i * SHARD:(i + 1) * SHARD])
        pt = (
            Pipad[i * SHARD:(i + 1) * SHARD, 0:K - 1]
            .reshape(CHUNKS, 128, K - 1)
            .transpose(1, 0, 2)
            .astype(ml_dtypes.bfloat16)
        )
        pb = np.ascontiguousarray(np.repeat(pt[..., None], 2, axis=-1))
        pf = np.ascontiguousarray(
            Pipad[i * SHARD:(i + 1) * SHARD, K - 1]
            .reshape(CHUNKS, 128)
            .T
        )
        in_maps.append({"Zs": zs, "Pb": pb, "Pf": pf})
    return in_maps


def _run_device(in_maps, trace=False, tmpdir=None):
    nc = _get_nc()
    return bass_utils.run_bass_kernel_spmd(
        nc, in_maps, core_ids=list(range(N_CORES)), trace=trace, tmpdir=tmpdir
    )


def _epilogue(G_all, Pi, A, U):
    """Host epilogue in float64. G_all: [128, 1408] summed partial Grams."""
    G_all = G_all.astype(np.float64)
    Gk = np.stack(
        [G_all[:, k * D:(k + 1) * D] for k in range(K)]
    )                                   # [K, D, D] masked Grams
    Gram = G_all[:, K * D:(K + 1) * D]  # [D, D] full Gram

    d_f = float(D)
    n_f = float(N)

    Mat = np.eye(D, dtype=np.float64) + (d_f / (n_f * EPS)) * Gram
    _, logdet = np.linalg.slogdet(Mat)
    loss_R = 0.5 * logdet

    trPi = Pi.astype(np.float64).sum(axis=0)            # [K]
    scalar = d_f / (trPi * EPS)
    Ar = np.maximum(A.astype(np.float64), 0.0)          # [K, D]
    logdets = np.log1p(scalar[:, None] * Ar).sum(axis=1)
    loss_Rc = 0.5 * np.sum(logdets * trPi) / n_f

    norms = np.maximum(np.linalg.norm(U, axis=0, keepdims=True), 1e-12)
    Un = (U / norms).astype(np.float64)
    M = np.einsum("dp,kp,ep->kde", Un, Ar, Un)
    loss_reg = 0.5 * np.sum((Gk - M) ** 2)

    loss_obj = loss_R - loss_Rc - MU * loss_reg
    return (
        np.float32(-loss_obj),
        np.float32(loss_R),
        np.float32(loss_Rc),
        np.float32(loss_reg),
    )


def kernel(Z, Pi, A, U):
    Z = np.asarray(Z, dtype=np.float32)
    Pi = np.asarray(Pi, dtype=np.float32)
    A = np.asarray(A, dtype=np.float32)
    U = np.asarray(U, dtype=np.float32)

    in_maps = _make_in_maps(Z, Pi)
    res = _run_device(in_maps)
    G_all = np.zeros((D, NCLS * D), np.float64)
    for i in range(N_CORES):
        G_all += res.results[i]["G"]
    return _epilogue(G_all, Pi, A, U)


# revision 21
# speedup vs baseline: 1.1113x; 1.0270x over previous
"""MCR2 variational loss on 8 Trainium2 NeuronCores.

Math (reference):
  loss_R   = 0.5 * logdet(I + d/(n*eps) * Z.T @ Z)
  loss_Rc  = 0.5 * sum_k(trPi_k * sum_p log1p(d/(trPi_k*eps) * relu(A)_kp)) / n
  loss_reg = 0.5 * sum_k || G_k - Un diag(relu(A)_k) Un.T ||_F^2,
             G_k = Z.T diag(Pi[:,k]) Z
  out = (-(loss_R - loss_Rc - mu*loss_reg), loss_R, loss_Rc, loss_reg)

The only O(n) work is the 11 Grams (10 masked + 1 full). Device strategy:
shard rows across 8 cores; per core, for each 128-row chunk of Z, build all
10 weighted copies W_k = Pi[:,k] * Z with a single DVE tensor_tensor whose
second operand is a broadcast AP over a bf16 [w,w] pair tile (pair packing
keeps the DVE 2x perf mode), then accumulate Z.T @ [W_0..W_9 | Z] into PSUM
on the tensor engine (bf16 x bf16 -> fp32). The fp32->bf16 cast of Z runs
on the scalar engine per 7-chunk group. Per-core partial Grams [128 x 1408]
are summed on host; the O(k*d^2) epilogue (slogdet etc.) runs on host in
float64.
"""

import sys

if "/opt/trn_rl_repo" not in sys.path:
    sys.path.insert(0, "/opt/trn_rl_repo")

import ml_dtypes
import numpy as np

import concourse.bacc as bacc
import concourse.mybir as mybir
import concourse.tile as tile
from concourse import bass_utils

# Problem constants (hardcoded per harness contract).
N, D, K = 100000, 128, 10
EPS, MU = 0.5, 1.0
N_CORES = 8
CHUNKS = 98                    # 128-row chunks per core
SHARD = CHUNKS * 128           # 12544 rows per core
NPAD = SHARD * N_CORES         # 100352 (zero-padded; zero rows contribute 0)
GROUP = 7                      # chunks per staged DMA group
N_GROUPS = CHUNKS // GROUP     # 14
NCLS = K + 1                   # 10 masked Grams + 1 full Gram

_NC_CACHE = None


def _build_nc():
    f32 = mybir.dt.float32
    bf16 = mybir.dt.bfloat16
    Copy = mybir.ActivationFunctionType.Copy

    nc = bacc.Bacc("TRN2", target_bir_lowering=False, debug=False)
    Zs = nc.dram_tensor("Zs", [SHARD, D], bf16, kind="ExternalInput")
    # Pi, host-preprocessed: [p, chunk, class, 2] bf16 with the weight
    # duplicated in the last axis so the DVE reads an aligned [w,w] pair.
    KD = K - 1  # classes 0..8 weighted on DVE; class 9 on ACT
    Pb = nc.dram_tensor("Pb", [128, CHUNKS, KD, 2], bf16, kind="ExternalInput")
    Pf = nc.dram_tensor("Pf", [128, CHUNKS], f32, kind="ExternalInput")
    G = nc.dram_tensor("G", [D, NCLS * D], f32, kind="ExternalOutput")

    with tile.TileContext(nc) as tc:
        with (
            tc.tile_pool(name="zbf", bufs=4) as zbpool,
            tc.tile_pool(name="wgt", bufs=3) as wpool,
            tc.tile_pool(name="pi", bufs=1) as pipool,
            tc.tile_pool(name="res", bufs=1) as opool,
            tc.tile_pool(name="warm", bufs=1) as warmpool,
            tc.tile_pool(name="ps", bufs=1, space="PSUM") as pspool,
        ):
            psA = pspool.tile([128, 512], f32, name="psA")
            psB = pspool.tile([128, 512], f32, name="psB")
            psC = pspool.tile([128, 256], f32, name="psC")
            psD = pspool.tile([128, 128], f32, name="psD")

            # PE warmup: dummy matmuls on scratch data keep the tensor
            # engine busy through the HAM activity window while the first
            # DMAs land, so real matmuls start at the full 2.4 GHz clock.
            wsrc = warmpool.tile([128, 256], bf16, name="wsrc")
            wps = pspool.tile([128, 256], f32, name="wps")
            nc.gpsimd.memset(wsrc[:], 0.0)
            for _ in range(56):
                nc.tensor.matmul(wps[:], wsrc[:, 0:128], wsrc[:], start=True,
                                 stop=True, skip_group_check=True)

            Zr = Zs.rearrange("(c p) d -> p c d", p=128)

            # First chunk's inputs first in the DMA queue, then the rest
            # of the Pi data, so compute starts as early as possible.
            pib = pipool.tile([128, CHUNKS, KD, 2], bf16, name="pib")
            pif = pipool.tile([128, CHUNKS], f32, name="pif")
            nc.sync.dma_start(pib[:, 0:1], Pb[:, 0:1])
            nc.sync.dma_start(pif[:, 0:1], Pf[:, 0:1])

            # Small first group so compute starts early; small last group
            # to shrink the pipeline drain.
            sizes = [1] + [GROUP] * 13 + [6]
            assert sum(sizes) == CHUNKS

            start_c = 0
            for gi, sz in enumerate(sizes):
                s0 = start_c
                start_c += sz
                zb = zbpool.tile([128, sz, D], bf16, name="zb", tag="zb")
                nc.sync.dma_start(zb[:], Zr[:, s0:s0 + sz, :])
                if gi == 0:
                    # rest of the Pi data, behind the first chunk's inputs
                    nc.sync.dma_start(pib[:, 1:CHUNKS], Pb[:, 1:CHUNKS])
                    nc.sync.dma_start(pif[:, 1:CHUNKS], Pf[:, 1:CHUNKS])

                # Fused weighted-copy for classes 0..8 over the whole group:
                #   wg[p, c, k, 2r+t] = zb[p, c, 2r+t] * pib[p, s0+c, k]
                # bf16 [w,w] pair packing keeps the DVE 2x perf mode.
                wg = wpool.tile([128, sz, K * D], bf16, name="wg", tag="wg")
                z_bc = zb[:].unsqueeze(2).broadcast_to([128, sz, KD, D])
                pi_bc = (
                    pib[:, s0:s0 + sz, :, :]
                    .unsqueeze(3)
                    .broadcast_to([128, sz, KD, 64, 2])
                )
                w5 = wg[:, :, 0:KD * D].rearrange(
                    "p c (k r t) -> p c k r t", k=KD, t=2
                )
                z5 = z_bc.rearrange("p c k (r t) -> p c k r t", t=2)
                nc.vector.tensor_mul(w5, z5, pi_bc)

                for c in range(sz):
                    idx = s0 + c
                    first = idx == 0
                    last = idx == CHUNKS - 1
                    zc = zb[:, c, :]
                    w = wg[:, c, :]
                    # class 9 weighted copy on the scalar engine
                    nc.scalar.activation(
                        w[:, KD * D:K * D], zc, Copy,
                        bias=0.0, scale=pif[:, idx:idx + 1],
                    )
                    nc.tensor.matmul(psA[:], zc, w[:, 0:512], start=first, stop=last)
                    nc.tensor.matmul(psB[:], zc, w[:, 512:1024], start=first, stop=last)
                    nc.tensor.matmul(psC[:], zc, w[:, 1024:1280], start=first, stop=last)
                    nc.tensor.matmul(psD[:], zc, zc, start=first, stop=last)

            out = opool.tile([128, NCLS * D], f32, name="out")
            nc.vector.tensor_copy(out[:, 0:512], psA[:])
            nc.scalar.copy(out[:, 512:1024], psB[:])
            nc.vector.tensor_copy(out[:, 1024:1280], psC[:])
            nc.scalar.copy(out[:, 1280:1408], psD[:])
            nc.sync.dma_start(G[:], out[:])

    nc.compile()
    return nc


def _get_nc():
    global _NC_CACHE
    if _NC_CACHE is None:
        _NC_CACHE = _build_nc()
    return _NC_CACHE


def _make_in_maps(Z, Pi):
    Zpad = np.zeros((NPAD, D), ml_dtypes.bfloat16)
    Zpad[:N] = Z.astype(ml_dtypes.bfloat16)
    Pipad = np.zeros((NPAD, K), np.float32)
    Pipad[:N] = Pi
    in_maps = []
    for i in range(N_CORES):
        zs = np.ascontiguousarray(Zpad[i * SHARD:(i + 1) * SHARD])
        pt = (
            Pipad[i * SHARD:(i + 1) * SHARD, 0:K - 1]
            .reshape(CHUNKS, 128, K - 1)
            .transpose(1, 0, 2)
            .astype(ml_dtypes.bfloat16)
        )
        pb = np.ascontiguousarray(np.repeat(pt[..., None], 2, axis=-1))
        pf = np.ascontiguousarray(
            Pipad[i * SHARD:(i + 1) * SHARD, K - 1]
            .reshape(CHUNKS, 128)
            .T
        )
        in_maps.append({"Zs": zs, "Pb": pb, "Pf": pf})
    return in_maps


def _run_device(in_maps, trace=False, tmpdir=None):
    nc = _get_nc()
    return bass_utils.run_bass_kernel_spmd(
        nc, in_maps, core_ids=list(range(N_CORES)), trace=trace, tmpdir=tmpdir
    )


def _epilogue(G_all, Pi, A, U):
    """Host epilogue in float64. G_all: [128, 1408] summed partial Grams."""
    G_all = G_all.astype(np.float64)
    Gk = np.stack(
        [G_all[:, k * D:(k + 1) * D] for k in range(K)]
    )                                   # [K, D, D] masked Grams
    Gram = G_all[:, K * D:(K + 1) * D]  # [D, D] full Gram

    d_f = float(D)
    n_f = float(N)

    Mat = np.eye(D, dtype=np.float64) + (d_f / (n_f * EPS)) * Gram
    _, logdet = np.linalg.slogdet(Mat)
    loss_R = 0.5 * logdet

    trPi = Pi.astype(np.float64).sum(axis=0)            # [K]
    scalar = d_f / (trPi * EPS)
    Ar = np.maximum(A.astype(np.float64), 0.0)          # [K, D]
    logdets = np.log1p(scalar[:, None] * Ar).sum(axis=1)
    loss_Rc = 0.5 * np.sum(logdets * trPi) / n_f

    norms = np.maximum(np.linalg.norm(U, axis=0, keepdims=True), 1e-12)
    Un = (U / norms).astype(np.float64)
    M = np.einsum("dp,kp,ep->kde", Un, Ar, Un)
    loss_reg = 0.5 * np.sum((Gk - M) ** 2)

    loss_obj = loss_R - loss_Rc - MU * loss_reg
    return (
        np.float32(-loss_obj),
        np.float32(loss_R),
        np.float32(loss_Rc),
        np.float32(loss_reg),
    )


def kernel(Z, Pi, A, U):
    Z = np.asarray(Z, dtype=np.float32)
    Pi = np.asarray(Pi, dtype=np.float32)
    A = np.asarray(A, dtype=np.float32)
    U = np.asarray(U, dtype=np.float32)

    in_maps = _make_in_maps(Z, Pi)
    res = _run_device(in_maps)
    G_all = np.zeros((D, NCLS * D), np.float64)
    for i in range(N_CORES):
        G_all += res.results[i]["G"]
    return _epilogue(G_all, Pi, A, U)


# revision 24
# speedup vs baseline: 1.1160x; 1.0042x over previous
"""MCR2 variational loss on 8 Trainium2 NeuronCores.

Math (reference):
  loss_R   = 0.5 * logdet(I + d/(n*eps) * Z.T @ Z)
  loss_Rc  = 0.5 * sum_k(trPi_k * sum_p log1p(d/(trPi_k*eps) * relu(A)_kp)) / n
  loss_reg = 0.5 * sum_k || G_k - Un diag(relu(A)_k) Un.T ||_F^2,
             G_k = Z.T diag(Pi[:,k]) Z
  out = (-(loss_R - loss_Rc - mu*loss_reg), loss_R, loss_Rc, loss_reg)

The only O(n) work is the 11 Grams (10 masked + 1 full). Device strategy:
shard rows across 8 cores; per core, for each 128-row chunk of Z, build all
10 weighted copies W_k = Pi[:,k] * Z with a single DVE tensor_tensor whose
second operand is a broadcast AP over a bf16 [w,w] pair tile (pair packing
keeps the DVE 2x perf mode), then accumulate Z.T @ [W_0..W_9 | Z] into PSUM
on the tensor engine (bf16 x bf16 -> fp32). The fp32->bf16 cast of Z runs
on the scalar engine per 7-chunk group. Per-core partial Grams [128 x 1408]
are summed on host; the O(k*d^2) epilogue (slogdet etc.) runs on host in
float64.
"""

import sys

if "/opt/trn_rl_repo" not in sys.path:
    sys.path.insert(0, "/opt/trn_rl_repo")

import ml_dtypes
import numpy as np

import concourse.bacc as bacc
import concourse.mybir as mybir
import concourse.tile as tile
from concourse import bass_utils

# Problem constants (hardcoded per harness contract).
N, D, K = 100000, 128, 10
EPS, MU = 0.5, 1.0
N_CORES = 8
CHUNKS = 98                    # 128-row chunks per core
SHARD = CHUNKS * 128           # 12544 rows per core
NPAD = SHARD * N_CORES         # 100352 (zero-padded; zero rows contribute 0)
GROUP = 7                      # chunks per staged DMA group
N_GROUPS = CHUNKS // GROUP     # 14
NCLS = K + 1                   # 10 masked Grams + 1 full Gram

_NC_CACHE = None


def _build_nc():
    f32 = mybir.dt.float32
    bf16 = mybir.dt.bfloat16
    Copy = mybir.ActivationFunctionType.Copy

    nc = bacc.Bacc("TRN2", target_bir_lowering=False, debug=False)
    Zs = nc.dram_tensor("Zs", [SHARD, D], bf16, kind="ExternalInput")
    # Pi, host-preprocessed: [p, chunk, class, 2] bf16 with the weight
    # duplicated in the last axis so the DVE reads an aligned [w,w] pair.
    KD = K - 1  # classes 0..8 weighted on DVE; class 9 on ACT
    Pb = nc.dram_tensor("Pb", [128, CHUNKS, KD, 2], bf16, kind="ExternalInput")
    Pf = nc.dram_tensor("Pf", [128, CHUNKS], f32, kind="ExternalInput")
    G = nc.dram_tensor("G", [D, NCLS * D], f32, kind="ExternalOutput")

    with tile.TileContext(nc) as tc:
        with (
            tc.tile_pool(name="zbf", bufs=4) as zbpool,
            tc.tile_pool(name="wgt", bufs=3) as wpool,
            tc.tile_pool(name="pi", bufs=1) as pipool,
            tc.tile_pool(name="res", bufs=1) as opool,
            tc.tile_pool(name="warm", bufs=1) as warmpool,
            tc.tile_pool(name="ps", bufs=1, space="PSUM") as pspool,
        ):
            psA = pspool.tile([128, 512], f32, name="psA")
            psB = pspool.tile([128, 512], f32, name="psB")
            psC = pspool.tile([128, 256], f32, name="psC")
            psD = pspool.tile([128, 128], f32, name="psD")

            # PE warmup: dummy matmuls on scratch data keep the tensor
            # engine busy through the HAM activity window while the first
            # DMAs land, so real matmuls start at the full 2.4 GHz clock.
            wsrc = warmpool.tile([128, 256], bf16, name="wsrc")
            wps = pspool.tile([128, 256], f32, name="wps")
            nc.gpsimd.memset(wsrc[:], 0.0)
            for _ in range(56):
                nc.tensor.matmul(wps[:], wsrc[:, 0:128], wsrc[:], start=True,
                                 stop=True, skip_group_check=True)

            Zr = Zs.rearrange("(c p) d -> p c d", p=128)

            # First chunk's inputs first in the DMA queue, then the rest
            # of the Pi data, so compute starts as early as possible.
            pib = pipool.tile([128, CHUNKS, KD, 2], bf16, name="pib")
            pif = pipool.tile([128, CHUNKS], f32, name="pif")
            # Pi loads on the (otherwise idle) gpsimd SWDGE queue so they
            # don't serialize behind the Z loads on the sync queue.
            nc.gpsimd.dma_start(pib[:, 0:1], Pb[:, 0:1])
            nc.gpsimd.dma_start(pif[:], Pf[:])
            nc.gpsimd.dma_start(pib[:, 1:CHUNKS], Pb[:, 1:CHUNKS])

            # Small first group so compute starts early; tapering last
            # groups to shrink the pipeline drain.
            sizes = [1] + [GROUP] * 12 + [5, 4, 3, 1]
            assert sum(sizes) == CHUNKS

            start_c = 0
            for gi, sz in enumerate(sizes):
                s0 = start_c
                start_c += sz
                zb = zbpool.tile([128, sz, D], bf16, name="zb", tag="zb")
                nc.sync.dma_start(zb[:], Zr[:, s0:s0 + sz, :])

                # Fused weighted-copy for classes 0..8 over the whole group:
                #   wg[p, c, k, 2r+t] = zb[p, c, 2r+t] * pib[p, s0+c, k]
                # bf16 [w,w] pair packing keeps the DVE 2x perf mode.
                wg = wpool.tile([128, sz, K * D], bf16, name="wg", tag="wg")
                z_bc = zb[:].unsqueeze(2).broadcast_to([128, sz, KD, D])
                pi_bc = (
                    pib[:, s0:s0 + sz, :, :]
                    .unsqueeze(3)
                    .broadcast_to([128, sz, KD, 64, 2])
                )
                w5 = wg[:, :, 0:KD * D].rearrange(
                    "p c (k r t) -> p c k r t", k=KD, t=2
                )
                z5 = z_bc.rearrange("p c k (r t) -> p c k r t", t=2)
                nc.vector.tensor_mul(w5, z5, pi_bc)

                for c in range(sz):
                    idx = s0 + c
                    first = idx == 0
                    last = idx == CHUNKS - 1
                    zc = zb[:, c, :]
                    w = wg[:, c, :]
                    # class 9 weighted copy on the scalar engine
                    nc.scalar.activation(
                        w[:, KD * D:K * D], zc, Copy,
                        bias=0.0, scale=pif[:, idx:idx + 1],
                    )
                    nc.tensor.matmul(psA[:], zc, w[:, 0:512], start=first, stop=last)
                    nc.tensor.matmul(psB[:], zc, w[:, 512:1024], start=first, stop=last)
                    nc.tensor.matmul(psC[:], zc, w[:, 1024:1280], start=first, stop=last)
                    nc.tensor.matmul(psD[:], zc, zc, start=first, stop=last)

            out = opool.tile([128, NCLS * D], f32, name="out")
            nc.vector.tensor_copy(out[:, 0:512], psA[:])
            nc.scalar.copy(out[:, 512:1024], psB[:])
            nc.vector.tensor_copy(out[:, 1024:1280], psC[:])
            nc.scalar.copy(out[:, 1280:1408], psD[:])
            nc.sync.dma_start(G[:], out[:])

    nc.compile()
    return nc


def _get_nc():
    global _NC_CACHE
    if _NC_CACHE is None:
        _NC_CACHE = _build_nc()
    return _NC_CACHE


def _make_in_maps(Z, Pi):
    Zpad = np.zeros((NPAD, D), ml_dtypes.bfloat16)
    Zpad[:N] = Z.astype(ml_dtypes.bfloat16)
    Pipad = np.zeros((NPAD, K), np.float32)
    Pipad[:N] = Pi
    in_maps = []
    for i in range(N_CORES):
        zs = np.ascontiguousarray(Zpad[i * SHARD:(i + 1) * SHARD])
        pt = (
            Pipad[i * SHARD:(i + 1) * SHARD, 0:K - 1]
            .reshape(CHUNKS, 128, K - 1)
            .transpose(1, 0, 2)
            .astype(ml_dtypes.bfloat16)
        )
        pb = np.ascontiguousarray(np.repeat(pt[..., None], 2, axis=-1))
        pf = np.ascontiguousarray(
            Pipad[i * SHARD:(i + 1) * SHARD, K - 1]
            .reshape(CHUNKS, 128)
            .T
        )
        in_maps.append({"Zs": zs, "Pb": pb, "Pf": pf})
    return in_maps


def _run_device(in_maps, trace=False, tmpdir=None):
    nc = _get_nc()
    return bass_utils.run_bass_kernel_spmd(
        nc, in_maps, core_ids=list(range(N_CORES)), trace=trace, tmpdir=tmpdir
    )


def _epilogue(G_all, Pi, A, U):
    """Host epilogue in float64. G_all: [128, 1408] summed partial Grams."""
    G_all = G_all.astype(np.float64)
    Gk = np.stack(
        [G_all[:, k * D:(k + 1) * D] for k in range(K)]
    )                                   # [K, D, D] masked Grams
    Gram = G_all[:, K * D:(K + 1) * D]  # [D, D] full Gram

    d_f = float(D)
    n_f = float(N)

    Mat = np.eye(D, dtype=np.float64) + (d_f / (n_f * EPS)) * Gram
    _, logdet = np.linalg.slogdet(Mat)
    loss_R = 0.5 * logdet

    trPi = Pi.astype(np.float64).sum(axis=0)            # [K]
    scalar = d_f / (trPi * EPS)
    Ar = np.maximum(A.astype(np.float64), 0.0)          # [K, D]
    logdets = np.log1p(scalar[:, None] * Ar).sum(axis=1)
    loss_Rc = 0.5 * np.sum(logdets * trPi) / n_f

    norms = np.maximum(np.linalg.norm(U, axis=0, keepdims=True), 1e-12)
    Un = (U / norms).astype(np.float64)
    M = np.einsum("dp,kp,ep->kde", Un, Ar, Un)
    loss_reg = 0.5 * np.sum((Gk - M) ** 2)

    loss_obj = loss_R - loss_Rc - MU * loss_reg
    return (
        np.float32(-loss_obj),
        np.float32(loss_R),
        np.float32(loss_Rc),
        np.float32(loss_reg),
    )


def kernel(Z, Pi, A, U):
    Z = np.asarray(Z, dtype=np.float32)
    Pi = np.asarray(Pi, dtype=np.float32)
    A = np.asarray(A, dtype=np.float32)
    U = np.asarray(U, dtype=np.float32)

    in_maps = _make_in_maps(Z, Pi)
    res = _run_device(in_maps)
    G_all = np.zeros((D, NCLS * D), np.float64)
    for i in range(N_CORES):
        G_all += res.results[i]["G"]
    return _epilogue(G_all, Pi, A, U)
